# revision 5
# baseline (speedup 1.0000x reference)
"""Trainium2 Bass kernel for AuxiliaryMultiHeadedAttention.

Reference computation (B=4, S=1024, HID=1024, H=16 heads, DH=64):
    qh  = split_heads(q @ Wq.T + bq)
    kh  = split_heads(k @ Wk.T + bk)
    vh  = split_heads(v @ Wv.T + bv)
    kbh = split_heads(k_b @ Wkb.T + bkb)
    corr = qh @ (kh + kbh).T / sqrt(3*DH)
    corr = where(mask[b, t] == 0, -1e9, corr)          # mask over key positions
    prob = softmax(corr, axis=-1)
    out  = merge_heads(prob @ vh) @ Wo.T + bo

Sharding: 8 cores = 4 batches x 2 head-groups (8 heads each).  Each core
computes its batch's projections for its 8 heads, attention, and a partial
output projection over its 512 hidden dims.  Host sums the two partials per
batch (replaces the all-reduce) and adds bo.

Device-side layout is feature-major ([feature, token]); the host feeds
pre-transposed activations and weights so no on-chip transposes are needed.
Scores are computed transposed ([t, s]); softmax over t is handled by
multiplying exp tiles against V extended with a mask column on the PE
(the 65th output row of the PV matmul is the softmax denominator), so no
partition-dim reductions are needed.  Matmul inputs are float32r by default
(full PE rate for fp32 data); KERNEL_MM_DT=bf16|f32 selects alternatives.
"""

import math
import os

import numpy as np

import concourse.bass as bass
import concourse.mybir as mybir
import concourse.tile as tile
from concourse import bacc
from concourse.bass_utils import run_bass_kernel_spmd

B, S, HID, H = 4, 1024, 1024, 16
DH = HID // H            # 64
NCORES = 8
HPC = H // 2             # 8 heads per core
DPC = HPC * DH           # 512 hidden dims per core
P = 128
KT = HID // P            # 8 k-tiles (contraction over hid)
ST = S // P              # 8 s/t-tiles
NB = 512                 # matmul moving free dim (one PSUM bank of fp32)
SC = S // NB             # 2 s-chunks
DT = DPC // P            # 4 d'-tiles
F32 = mybir.dt.float32
SCALE = 1.0 / math.sqrt(3 * DH)

_MM_NAME = os.environ.get("KERNEL_MM_DT", "f32r")
MM_DT = {
    "f32r": mybir.dt.float32r,
    "bf16": mybir.dt.bfloat16,
    "f32": mybir.dt.float32,
}[_MM_NAME]


def _np_mm_dt():
    if _MM_NAME == "bf16":
        import ml_dtypes
        return ml_dtypes.bfloat16
    return np.float32


def build_module():
    nc = bacc.Bacc(
        "TRN2",
        target_bir_lowering=False,
        debug=False,
        num_devices=NCORES,
    )
    io = {}

    def din(name, shape, dt=MM_DT):
        io[name] = nc.dram_tensor(name, shape, dt, kind="ExternalInput").ap()

    din("qT", [HID, S])
    din("kT", [HID, S])
    din("kbT", [HID, S])
    din("vT", [HID, S])
    din("wqT", [HID, DPC])
    din("wkT", [HID, DPC])
    din("wkbT", [HID, DPC])
    din("wvT", [HID, DPC])
    din("woT", [DPC, HID])
    din("bq", [DPC], F32)
    din("bks", [DPC], F32)    # bk + bkb, summed on host
    din("bv", [DPC])          # matmul rhs (K=1 bias trick) -> MM_DT
    din("maskf", [S], F32)    # mask[b] as float 0/1
    din("onesd", [P])         # constant ones row -> MM_DT
    io["out"] = nc.dram_tensor("out", [S, HID], F32, kind="ExternalOutput").ap()

    with tile.TileContext(nc) as tc:
        _build_kernel(tc, io)
    nc.compile()
    return nc


def _build_kernel(tc, io):
    from contextlib import ExitStack

    nc = tc.nc
    Exp = mybir.ActivationFunctionType.Exp

    with ExitStack() as ctx:
        ctx.enter_context(
            nc.allow_low_precision(reason="matmul inputs intentionally MM_DT")
        )
        singles = ctx.enter_context(tc.tile_pool(name="singles", bufs=1))
        wts = ctx.enter_context(tc.tile_pool(name="wts", bufs=2))
        acts = ctx.enter_context(tc.tile_pool(name="acts", bufs=3))
        expp = ctx.enter_context(tc.tile_pool(name="expp", bufs=2))
        outp = ctx.enter_context(tc.tile_pool(name="outp", bufs=3))
        smalls = ctx.enter_context(tc.tile_pool(name="smalls", bufs=4))
        ps_sc = ctx.enter_context(tc.tile_pool(name="ps_sc", bufs=2, space="PSUM"))
        ps_acc = ctx.enter_context(tc.tile_pool(name="ps_acc", bufs=4, space="PSUM"))

        # Resident intermediates, feature-major.  All matmul inputs use MM_DT.
        QHT = singles.tile([P, DT, S], MM_DT, tag="qht")      # qh.T   [d', s]
        KSUMT = singles.tile([P, DT, S], MM_DT, tag="ksumt")  # (kh+kbh).T [d', t]
        # V + mask column, token-major: per t-tile, per head: 64 vh cols + mask
        VHM = singles.tile([P, ST, HPC, DH + 1], MM_DT, tag="vhm")
        HT = singles.tile([P, DT, S], MM_DT, tag="ht")        # hidden.T [d', s]

        # Constants
        bq_s = singles.tile([P, DT], F32, tag="bq")
        bks_s = singles.tile([P, DT], F32, tag="bks")
        mask_c = singles.tile([P, ST], F32, tag="mask")
        bv_row = singles.tile([1, DPC], MM_DT, tag="bv")
        ones1 = singles.tile([1, P], MM_DT, tag="ones")

        nc.sync.dma_start(bq_s, io["bq"].rearrange("(t p) -> p t", p=P))
        nc.sync.dma_start(bks_s, io["bks"].rearrange("(t p) -> p t", p=P))
        nc.sync.dma_start(mask_c, io["maskf"].rearrange("(t p) -> p t", p=P))
        nc.sync.dma_start(bv_row, io["bv"].rearrange("(o d) -> o d", o=1))
        nc.sync.dma_start(ones1, io["onesd"].rearrange("(o d) -> o d", o=1))

        def act_chunk(name, c):
            t = acts.tile([P, KT, NB], MM_DT, tag="act")
            src = io[name].rearrange("(kt p) s -> p kt s", p=P)
            nc.sync.dma_start(t, src[:, :, c * NB:(c + 1) * NB])
            return t

        def load_w(name):
            w = wts.tile([P, KT, DPC], MM_DT, tag="w")
            nc.sync.dma_start(w, io[name].rearrange("(kt p) m -> p kt m", p=P))
            return w

        # ---- Stage A1: QHT[d', s] = (Wq_g @ q.T) + bq ----
        wq = load_w("wqT")
        for c in range(SC):
            qc = act_chunk("qT", c)
            for dt_ in range(DT):
                ps = ps_acc.tile([P, NB], F32, tag="ps1")
                for kt in range(KT):
                    nc.tensor.matmul(
                        ps,
                        lhsT=wq[:, kt, dt_ * P:(dt_ + 1) * P],
                        rhs=qc[:, kt, :],
                        start=(kt == 0),
                        stop=(kt == KT - 1),
                    )
                nc.vector.tensor_scalar_add(
                    QHT[:, dt_, c * NB:(c + 1) * NB], ps, bq_s[:, dt_:dt_ + 1]
                )

        # ---- Stage A2: KSUMT[d', t] = Wk_g @ k.T + Wkb_g @ k_b.T + bks ----
        wk = load_w("wkT")
        wkb = load_w("wkbT")
        for c in range(SC):
            kc = act_chunk("kT", c)
            kbc = act_chunk("kbT", c)
            for dt_ in range(DT):
                ps = ps_acc.tile([P, NB], F32, tag="ps1")
                for kt in range(KT):
                    nc.tensor.matmul(
                        ps,
                        lhsT=wk[:, kt, dt_ * P:(dt_ + 1) * P],
                        rhs=kc[:, kt, :],
                        start=(kt == 0),
                        stop=False,
                    )
                for kt in range(KT):
                    nc.tensor.matmul(
                        ps,
                        lhsT=wkb[:, kt, dt_ * P:(dt_ + 1) * P],
                        rhs=kbc[:, kt, :],
                        start=False,
                        stop=(kt == KT - 1),
                    )
                nc.vector.tensor_scalar_add(
                    KSUMT[:, dt_, c * NB:(c + 1) * NB], ps, bks_s[:, dt_:dt_ + 1]
                )

        # ---- Stage A3: VHM[t, h, 0:64] = (v.T_tile.T @ Wv.T + bv) * mask[t];
        #      VHM[t, h, 64] = mask[t] ----
        wv = load_w("wvT")
        for c in range(SC):
            vc = act_chunk("vT", c)
            for tl in range(ST // SC):
                tt = c * (ST // SC) + tl
                ps = ps_acc.tile([P, NB], F32, tag="ps1")
                for kt in range(KT):
                    nc.tensor.matmul(
                        ps,
                        lhsT=vc[:, kt, tl * P:(tl + 1) * P],
                        rhs=wv[:, kt, :],
                        start=(kt == 0),
                        stop=False,
                    )
                # bias along the free (d') dim via a K=1 ones matmul
                nc.tensor.matmul(
                    ps,
                    lhsT=ones1,
                    rhs=bv_row,
                    start=False,
                    stop=True,
                )
                nc.vector.tensor_scalar_mul(
                    VHM[:, tt, :, 0:DH],
                    ps.rearrange("p (h d) -> p h d", h=HPC),
                    mask_c[:, tt:tt + 1],
                )
                nc.vector.tensor_copy(
                    VHM[:, tt, :, DH:DH + 1],
                    mask_c[:, tt:tt + 1, None].to_broadcast((P, HPC, 1)),
                )

        # ---- Stage B: per (head, s-chunk) attention ----
        for h in range(HPC):
            r = h // 2
            bp = (h % 2) * DH
            for c in range(SC):
                ex = expp.tile([P, ST, NB], MM_DT, tag="exp")
                for jj in range(ST // 2):
                    ps2 = ps_sc.tile([P, 2, NB], F32, tag="ps2")
                    for u in range(2):
                        j = jj * 2 + u
                        nc.tensor.matmul(
                            ps2[:, u],
                            lhsT=KSUMT[bp:bp + DH, r, j * P:(j + 1) * P],
                            rhs=QHT[bp:bp + DH, r, c * NB:(c + 1) * NB],
                            start=True,
                            stop=True,
                        )
                    nc.scalar.activation(
                        ex[:, jj * 2:(jj + 1) * 2, :], ps2, Exp,
                        bias=0.0, scale=SCALE,
                    )
                # PV with fused denominator (65th row = sum_t exp * mask)
                psh = ps_acc.tile([P, NB], F32, tag="ps1")
                for j in range(ST):
                    nc.tensor.matmul(
                        psh[0:DH + 1, :],
                        lhsT=VHM[:, j, h, :],
                        rhs=ex[:, j, :],
                        start=(j == 0),
                        stop=(j == ST - 1),
                    )
                rec = smalls.tile([1, NB], MM_DT, tag="rec")
                nc.vector.reciprocal(rec, psh[DH:DH + 1, :])
                # broadcast the reciprocal across 64 partitions on the PE
                psb = ps_acc.tile([P, NB], F32, tag="ps1")
                nc.tensor.matmul(
                    psb[0:DH, :],
                    lhsT=ones1[:, 0:DH],
                    rhs=rec,
                    start=True,
                    stop=True,
                )
                recb = smalls.tile([DH, NB], F32, tag="recb")
                nc.vector.tensor_copy(recb, psb[0:DH, :])
                nc.vector.tensor_mul(
                    HT[bp:bp + DH, r, c * NB:(c + 1) * NB],
                    psh[0:DH, :],
                    recb,
                )

        # ---- Stage C: out[s, :] = hidden.T.T @ Wo_g.T (partial; host adds bo) ----
        wo = wts.tile([P, DT, HID], MM_DT, tag="w")
        nc.sync.dma_start(wo, io["woT"].rearrange("(it p) j -> p it j", p=P))
        for mt in range(ST):
            for c2 in range(SC):
                ps = ps_acc.tile([P, NB], F32, tag="ps1")
                for it in range(DT):
                    nc.tensor.matmul(
                        ps,
                        lhsT=HT[:, it, mt * P:(mt + 1) * P],
                        rhs=wo[:, it, c2 * NB:(c2 + 1) * NB],
                        start=(it == 0),
                        stop=(it == DT - 1),
                    )
                ot = outp.tile([P, NB], F32, tag="ot")
                nc.vector.tensor_copy(ot, ps)
                nc.sync.dma_start(
                    io["out"][mt * P:(mt + 1) * P, c2 * NB:(c2 + 1) * NB], ot
                )


def make_in_maps(inputs):
    inp = {k: np.asarray(v) for k, v in inputs.items()}
    q, k, v, k_b = inp["q"], inp["k"], inp["v"], inp["k_b"]
    mask = inp["mask"]
    f32 = np.float32
    mdt = _np_mm_dt()
    in_maps = []
    for core in range(NCORES):
        b, g = divmod(core, 2)
        hs = slice(g * DPC, (g + 1) * DPC)
        in_maps.append({
            "qT": np.ascontiguousarray(q[b].T).astype(mdt),
            "kT": np.ascontiguousarray(k[b].T).astype(mdt),
            "kbT": np.ascontiguousarray(k_b[b].T).astype(mdt),
            "vT": np.ascontiguousarray(v[b].T).astype(mdt),
            "wqT": np.ascontiguousarray(inp["Wq"][hs, :].T).astype(mdt),
            "wkT": np.ascontiguousarray(inp["Wk"][hs, :].T).astype(mdt),
            "wkbT": np.ascontiguousarray(inp["Wkb"][hs, :].T).astype(mdt),
            "wvT": np.ascontiguousarray(inp["Wv"][hs, :].T).astype(mdt),
            "woT": np.ascontiguousarray(inp["Wo"][:, hs].T).astype(mdt),
            "bq": np.ascontiguousarray(inp["bq"][hs], dtype=f32),
            "bks": np.ascontiguousarray(inp["bk"][hs] + inp["bkb"][hs], dtype=f32),
            "bv": np.ascontiguousarray(inp["bv"][hs]).astype(mdt),
            "maskf": mask[b].astype(f32),
            "onesd": np.ones(P, dtype=f32).astype(mdt),
        })
    return in_maps


def gather(results, bo):
    out = np.empty((B, S, HID), np.float32)
    bo = np.asarray(bo, dtype=np.float32)
    for b in range(B):
        out[b] = results[2 * b]["out"] + results[2 * b + 1]["out"] + bo
    return out


_module = None


def get_module():
    global _module
    if _module is None:
        _module = build_module()
    return _module


def kernel(**inputs):
    nc = get_module()
    in_maps = make_in_maps(inputs)
    res = run_bass_kernel_spmd(nc, in_maps, core_ids=list(range(NCORES))).results
    return gather(res, inputs["bo"])


# revision 6
# speedup vs baseline: 1.2711x; 1.2711x over previous
"""Trainium2 Bass kernel for AuxiliaryMultiHeadedAttention.

Reference computation (B=4, S=1024, HID=1024, H=16 heads, DH=64):
    qh  = split_heads(q @ Wq.T + bq)
    kh  = split_heads(k @ Wk.T + bk)
    vh  = split_heads(v @ Wv.T + bv)
    kbh = split_heads(k_b @ Wkb.T + bkb)
    corr = qh @ (kh + kbh).T / sqrt(3*DH)
    corr = where(mask[b, t] == 0, -1e9, corr)          # mask over key positions
    prob = softmax(corr, axis=-1)
    out  = merge_heads(prob @ vh) @ Wo.T + bo

Sharding: 8 cores = 4 batches x 2 head-groups (8 heads each).  Each core
computes its batch's projections for its 8 heads, attention, and a partial
output projection over its 512 hidden dims.  Host sums the two partials per
batch (replaces the all-reduce) and adds bo.

Device-side layout is feature-major ([feature, token]); the host feeds
pre-transposed activations and weights so no on-chip transposes are needed.
Scores are computed transposed ([t, s]); softmax over t is handled by
multiplying exp tiles against V extended with a mask column on the PE
(the 65th output row of the PV matmul is the softmax denominator), so no
partition-dim reductions are needed.  Matmul inputs are float32r by default
(full PE rate for fp32 data); KERNEL_MM_DT=bf16|f32 selects alternatives.
"""

import math
import os

import numpy as np

import concourse.bass as bass
import concourse.mybir as mybir
import concourse.tile as tile
from concourse import bacc
from concourse.bass_utils import run_bass_kernel_spmd

B, S, HID, H = 4, 1024, 1024, 16
DH = HID // H            # 64
NCORES = 8
HPC = H // 2             # 8 heads per core
DPC = HPC * DH           # 512 hidden dims per core
P = 128
KT = HID // P            # 8 k-tiles (contraction over hid)
ST = S // P              # 8 s/t-tiles
NB = 512                 # matmul moving free dim (one PSUM bank of fp32)
SC = S // NB             # 2 s-chunks
DT = DPC // P            # 4 d'-tiles
F32 = mybir.dt.float32
SCALE = 1.0 / math.sqrt(3 * DH)

_MM_NAME = os.environ.get("KERNEL_MM_DT", "f32r")
REPS_IN_NEFF = int(os.environ.get("KERNEL_REPS", "1"))
MM_DT = {
    "f32r": mybir.dt.float32r,
    "bf16": mybir.dt.bfloat16,
    "f32": mybir.dt.float32,
}[_MM_NAME]


def _np_mm_dt():
    if _MM_NAME == "bf16":
        import ml_dtypes
        return ml_dtypes.bfloat16
    return np.float32


def build_module():
    nc = bacc.Bacc(
        "TRN2",
        target_bir_lowering=False,
        debug=False,
        num_devices=NCORES,
    )
    io = {}

    def din(name, shape, dt=MM_DT):
        io[name] = nc.dram_tensor(name, shape, dt, kind="ExternalInput").ap()

    din("qT", [HID, S])
    din("kT", [HID, S])
    din("kbT", [HID, S])
    din("vT", [HID, S])
    din("wqT", [HID, DPC])
    din("wkT", [HID, DPC])
    din("wkbT", [HID, DPC])
    din("wvT", [HID, DPC])
    din("woT", [DPC, HID])
    din("bq", [DPC], F32)
    din("bks", [DPC], F32)    # bk + bkb, summed on host
    din("bv", [DPC])          # matmul rhs (K=1 bias trick) -> MM_DT
    din("maskf", [S], F32)    # mask[b] as float 0/1
    din("onesd", [P])         # constant ones row -> MM_DT
    io["out"] = nc.dram_tensor("out", [S, HID], F32, kind="ExternalOutput").ap()

    with tile.TileContext(nc) as tc:
        _build_kernel(tc, io)
    nc.compile()
    return nc


def _build_kernel(tc, io):
    from contextlib import ExitStack

    nc = tc.nc
    Exp = mybir.ActivationFunctionType.Exp

    with ExitStack() as ctx:
        ctx.enter_context(
            nc.allow_low_precision(reason="matmul inputs intentionally MM_DT")
        )
        singles = ctx.enter_context(tc.tile_pool(name="singles", bufs=1))
        wts = ctx.enter_context(tc.tile_pool(name="wts", bufs=2))
        acts = ctx.enter_context(tc.tile_pool(name="acts", bufs=3))
        expp = ctx.enter_context(tc.tile_pool(name="expp", bufs=2))
        outp = ctx.enter_context(tc.tile_pool(name="outp", bufs=3))
        smalls = ctx.enter_context(tc.tile_pool(name="smalls", bufs=4))
        ps_sc = ctx.enter_context(tc.tile_pool(name="ps_sc", bufs=2, space="PSUM"))
        ps_acc = ctx.enter_context(tc.tile_pool(name="ps_acc", bufs=4, space="PSUM"))

        # Resident intermediates, feature-major.  All matmul inputs use MM_DT.
        QHT = singles.tile([P, DT, S], MM_DT, tag="qht")      # qh.T   [d', s]
        KSUMT = singles.tile([P, DT, S], MM_DT, tag="ksumt")  # (kh+kbh).T [d', t]
        # V + mask column, token-major: per t-tile, per head: 64 vh cols + mask
        VHM = singles.tile([P, ST, HPC, DH + 1], MM_DT, tag="vhm")
        HT = singles.tile([P, DT, S], MM_DT, tag="ht")        # hidden.T [d', s]

        # Constants
        bq_s = singles.tile([P, DT], F32, tag="bq")
        bks_s = singles.tile([P, DT], F32, tag="bks")
        mask_c = singles.tile([P, ST], F32, tag="mask")
        bv_row = singles.tile([1, DPC], MM_DT, tag="bv")
        ones1 = singles.tile([1, P], MM_DT, tag="ones")

        nc.sync.dma_start(bq_s, io["bq"].rearrange("(t p) -> p t", p=P))
        nc.sync.dma_start(bks_s, io["bks"].rearrange("(t p) -> p t", p=P))
        nc.sync.dma_start(mask_c, io["maskf"].rearrange("(t p) -> p t", p=P))
        nc.sync.dma_start(bv_row, io["bv"].rearrange("(o d) -> o d", o=1))
        nc.sync.dma_start(ones1, io["onesd"].rearrange("(o d) -> o d", o=1))

        for _rep in range(REPS_IN_NEFF):
            _build_body(tc, io, locals())


def _build_body(tc, io, env):
    nc = tc.nc
    Exp = mybir.ActivationFunctionType.Exp
    singles = env["singles"]; wts = env["wts"]; acts = env["acts"]
    expp = env["expp"]; outp = env["outp"]; smalls = env["smalls"]
    ps_sc = env["ps_sc"]; ps_acc = env["ps_acc"]
    QHT = env["QHT"]; KSUMT = env["KSUMT"]; VHM = env["VHM"]; HT = env["HT"]
    bq_s = env["bq_s"]; bks_s = env["bks_s"]; mask_c = env["mask_c"]
    bv_row = env["bv_row"]; ones1 = env["ones1"]
    if True:

        def act_chunk(name, c):
            t = acts.tile([P, KT, NB], MM_DT, tag="act")
            src = io[name].rearrange("(kt p) s -> p kt s", p=P)
            nc.sync.dma_start(t, src[:, :, c * NB:(c + 1) * NB])
            return t

        def load_w(name):
            w = wts.tile([P, KT, DPC], MM_DT, tag="w")
            nc.sync.dma_start(w, io[name].rearrange("(kt p) m -> p kt m", p=P))
            return w

        # ---- Stage A1: QHT[d', s] = (Wq_g @ q.T) + bq ----
        wq = load_w("wqT")
        for c in range(SC):
            qc = act_chunk("qT", c)
            for dt_ in range(DT):
                ps = ps_acc.tile([P, NB], F32, tag="ps1")
                for kt in range(KT):
                    nc.tensor.matmul(
                        ps,
                        lhsT=wq[:, kt, dt_ * P:(dt_ + 1) * P],
                        rhs=qc[:, kt, :],
                        start=(kt == 0),
                        stop=(kt == KT - 1),
                    )
                nc.vector.tensor_scalar_add(
                    QHT[:, dt_, c * NB:(c + 1) * NB], ps, bq_s[:, dt_:dt_ + 1]
                )

        # ---- Stage A2: KSUMT[d', t] = Wk_g @ k.T + Wkb_g @ k_b.T + bks ----
        wk = load_w("wkT")
        wkb = load_w("wkbT")
        for c in range(SC):
            kc = act_chunk("kT", c)
            kbc = act_chunk("kbT", c)
            for dt_ in range(DT):
                ps = ps_acc.tile([P, NB], F32, tag="ps1")
                for kt in range(KT):
                    nc.tensor.matmul(
                        ps,
                        lhsT=wk[:, kt, dt_ * P:(dt_ + 1) * P],
                        rhs=kc[:, kt, :],
                        start=(kt == 0),
                        stop=False,
                    )
                for kt in range(KT):
                    nc.tensor.matmul(
                        ps,
                        lhsT=wkb[:, kt, dt_ * P:(dt_ + 1) * P],
                        rhs=kbc[:, kt, :],
                        start=False,
                        stop=(kt == KT - 1),
                    )
                nc.vector.tensor_scalar_add(
                    KSUMT[:, dt_, c * NB:(c + 1) * NB], ps, bks_s[:, dt_:dt_ + 1]
                )

        # ---- Stage A3: VHM[t, h, 0:64] = (v.T_tile.T @ Wv.T + bv) * mask[t];
        #      VHM[t, h, 64] = mask[t] ----
        wv = load_w("wvT")
        for c in range(SC):
            vc = act_chunk("vT", c)
            for tl in range(ST // SC):
                tt = c * (ST // SC) + tl
                ps = ps_acc.tile([P, NB], F32, tag="ps1")
                for kt in range(KT):
                    nc.tensor.matmul(
                        ps,
                        lhsT=vc[:, kt, tl * P:(tl + 1) * P],
                        rhs=wv[:, kt, :],
                        start=(kt == 0),
                        stop=False,
                    )
                # bias along the free (d') dim via a K=1 ones matmul
                nc.tensor.matmul(
                    ps,
                    lhsT=ones1,
                    rhs=bv_row,
                    start=False,
                    stop=True,
                )
                nc.vector.tensor_scalar_mul(
                    VHM[:, tt, :, 0:DH],
                    ps.rearrange("p (h d) -> p h d", h=HPC),
                    mask_c[:, tt:tt + 1],
                )
                nc.vector.tensor_copy(
                    VHM[:, tt, :, DH:DH + 1],
                    mask_c[:, tt:tt + 1, None].to_broadcast((P, HPC, 1)),
                )

        # ---- Stage B: per (head, s-chunk) attention ----
        for h in range(HPC):
            r = h // 2
            bp = (h % 2) * DH
            for c in range(SC):
                ex = expp.tile([P, ST, NB], MM_DT, tag="exp")
                for jj in range(ST // 2):
                    ps2 = ps_sc.tile([P, 2, NB], F32, tag="ps2")
                    for u in range(2):
                        j = jj * 2 + u
                        nc.tensor.matmul(
                            ps2[:, u],
                            lhsT=KSUMT[bp:bp + DH, r, j * P:(j + 1) * P],
                            rhs=QHT[bp:bp + DH, r, c * NB:(c + 1) * NB],
                            start=True,
                            stop=True,
                        )
                    nc.scalar.activation(
                        ex[:, jj * 2:(jj + 1) * 2, :], ps2, Exp,
                        bias=0.0, scale=SCALE,
                    )
                # PV with fused denominator (65th row = sum_t exp * mask)
                psh = ps_acc.tile([P, NB], F32, tag="ps1")
                for j in range(ST):
                    nc.tensor.matmul(
                        psh[0:DH + 1, :],
                        lhsT=VHM[:, j, h, :],
                        rhs=ex[:, j, :],
                        start=(j == 0),
                        stop=(j == ST - 1),
                    )
                rec = smalls.tile([1, NB], MM_DT, tag="rec")
                nc.vector.reciprocal(rec, psh[DH:DH + 1, :])
                # broadcast the reciprocal across 64 partitions on the PE
                psb = ps_acc.tile([P, NB], F32, tag="ps1")
                nc.tensor.matmul(
                    psb[0:DH, :],
                    lhsT=ones1[:, 0:DH],
                    rhs=rec,
                    start=True,
                    stop=True,
                )
                recb = smalls.tile([DH, NB], F32, tag="recb")
                nc.vector.tensor_copy(recb, psb[0:DH, :])
                nc.vector.tensor_mul(
                    HT[bp:bp + DH, r, c * NB:(c + 1) * NB],
                    psh[0:DH, :],
                    recb,
                )

        # ---- Stage C: out[s, :] = hidden.T.T @ Wo_g.T (partial; host adds bo) ----
        wo = wts.tile([P, DT, HID], MM_DT, tag="w")
        nc.sync.dma_start(wo, io["woT"].rearrange("(it p) j -> p it j", p=P))
        for mt in range(ST):
            for c2 in range(SC):
                ps = ps_acc.tile([P, NB], F32, tag="ps1")
                for it in range(DT):
                    nc.tensor.matmul(
                        ps,
                        lhsT=HT[:, it, mt * P:(mt + 1) * P],
                        rhs=wo[:, it, c2 * NB:(c2 + 1) * NB],
                        start=(it == 0),
                        stop=(it == DT - 1),
                    )
                ot = outp.tile([P, NB], F32, tag="ot")
                nc.vector.tensor_copy(ot, ps)
                nc.sync.dma_start(
                    io["out"][mt * P:(mt + 1) * P, c2 * NB:(c2 + 1) * NB], ot
                )


def make_in_maps(inputs):
    inp = {k: np.asarray(v) for k, v in inputs.items()}
    q, k, v, k_b = inp["q"], inp["k"], inp["v"], inp["k_b"]
    mask = inp["mask"]
    f32 = np.float32
    mdt = _np_mm_dt()
    in_maps = []
    for core in range(NCORES):
        b, g = divmod(core, 2)
        hs = slice(g * DPC, (g + 1) * DPC)
        in_maps.append({
            "qT": np.ascontiguousarray(q[b].T).astype(mdt),
            "kT": np.ascontiguousarray(k[b].T).astype(mdt),
            "kbT": np.ascontiguousarray(k_b[b].T).astype(mdt),
            "vT": np.ascontiguousarray(v[b].T).astype(mdt),
            "wqT": np.ascontiguousarray(inp["Wq"][hs, :].T).astype(mdt),
            "wkT": np.ascontiguousarray(inp["Wk"][hs, :].T).astype(mdt),
            "wkbT": np.ascontiguousarray(inp["Wkb"][hs, :].T).astype(mdt),
            "wvT": np.ascontiguousarray(inp["Wv"][hs, :].T).astype(mdt),
            "woT": np.ascontiguousarray(inp["Wo"][:, hs].T).astype(mdt),
            "bq": np.ascontiguousarray(inp["bq"][hs], dtype=f32),
            "bks": np.ascontiguousarray(inp["bk"][hs] + inp["bkb"][hs], dtype=f32),
            "bv": np.ascontiguousarray(inp["bv"][hs]).astype(mdt),
            "maskf": mask[b].astype(f32),
            "onesd": np.ones(P, dtype=f32).astype(mdt),
        })
    return in_maps


def gather(results, bo):
    out = np.empty((B, S, HID), np.float32)
    bo = np.asarray(bo, dtype=np.float32)
    for b in range(B):
        out[b] = results[2 * b]["out"] + results[2 * b + 1]["out"] + bo
    return out


_module = None


def get_module():
    global _module
    if _module is None:
        _module = build_module()
    return _module


def kernel(**inputs):
    nc = get_module()
    in_maps = make_in_maps(inputs)
    res = run_bass_kernel_spmd(nc, in_maps, core_ids=list(range(NCORES))).results
    return gather(res, inputs["bo"])
